# revision 28
# baseline (speedup 1.0000x reference)
"""Trainium2 Bass kernel for nn_BasicBlock_1w4a_LUT (binarized 3x3 conv + LUT bucketize).

Data-parallel over batch: 8 NeuronCores x 4 images each; no cross-core
communication. Full inputs in, full output out; shard/unshard on the host.

v2 — 4-quadrant PE tiling.

Host prep:
  - Binarize weights exactly as the reference; device weights are exactly
    +/-1 in fp16 with the pow2 scale folded into the LUT thresholds.
  - x is sent as plain fp16 (no hi/lo split: measured rel err 5e-3 << 2e-2
    budget). Each image is zero-padded into a flat 114x114 slab. A "pair
    slab" holds image A on partitions 0-63 and image B on partitions
    64-127, so the two PE row-groups work on different images and the
    input is DMA'd exactly once (halves HBM traffic vs hi/lo).
  - Per-channel affine z = y*s + b maps thresholds tau3 -> 0, tau5 -> 1
    (frees DVE scalar slots; 3 per-partition slots per custom op).

Device, per image pair:
  - The 3x3/pad-1 conv is 9 shifted dot products over the flat slab
    (junk at the 2-col row seams is computed and stripped on host).
    Output is processed in 26 chunks of 512 flat positions per image.
  - The 128x128 PE runs as FOUR 64x64 quadrant tiles (tile_position):
    row tile = image (A/B), col tile = chunk parity. Each tap round
    issues 4 concurrent K=64 matmuls, so the array is fully utilized
    with single-copy fp16 inputs — 2x the column throughput of a
    K=128 hi/lo scheme.
  - Per group (2 chunks x 2 images) the 9 tap rounds accumulate into one
    [128, 1024] PSUM tile (2 banks: img A | img B); ScalarE applies the
    per-channel affine out of PSUM in one FD=1024 pass writing z fp16.
  - Two custom DVE ops (BUCKET3 / BUCKET4ACC, registered at import into
    concourse's custom-DVE table) compute out = sum_k [z > tau_k] over
    all 7 thresholds in 2 passes per block (1-4 group blocks, small first
    for early pipeline start), writing u8. The Vector engine stream is the
    overall bottleneck (~64us of the ~86us span).
  - Input DMA pieces not needed until later groups are deferred behind
    marker-copy dependencies: every in-flight DMA counts toward the shared
    DMA-completion semaphore, so early consumers would otherwise gate on
    pair-1's transfers (~10us of extra startup latency).
"""

import numpy as np

# ---- problem constants (hardcoded per contract) ----
B, Cin, Cout, H, W = 32, 64, 64, 112, 112
NCORES = 8
BPC = B // NCORES          # images per core
NPAIR = BPC // 2           # image pairs per core (2)
HP = H + 2                 # 114
WPAD = W + 2               # 114
FLAT = HP * WPAD           # 12996 real slab elements
SLAB = 13568               # padded slab alloc (max rhs read 13311+230)
NCH = 512                  # flat positions per chunk (one PSUM bank)
NCHUNK = 26                # chunks per image (26*512 = 13312 >= 12766)
FOUT = NCHUNK * NCH        # 13312 flat out positions per image
NG = NCHUNK // 2           # 13 groups per pair (2 chunks of each image)
NTAPS = 9
NWARM = 9                  # PE warm-up matmuls
BLOCK_ENDS = [1, 2, 3, 5, 7, 9, 13]  # DVE block group-boundaries per pair
VALID = (H - 1) * WPAD + W # 12766 flat positions that contain real pixels

_built = []
last_results = None


def _register_dve_ops():
    from concourse.dve_spec import (
        Spec, Src0, Src1, C0, C1, C3, One, Zero, lower,
        _spill_c3_to_src1, _has_src1,
    )
    import concourse.dve_ops as dve_ops
    from concourse.dve_ops import DveOp
    from concourse.dve_uop import DveOpSpec

    def register_op(name, spec):
        if name in dve_ops._SUB_OPCODE_FOR_NAME:
            for op in dve_ops.OPS:
                if op.name == name:
                    return op
            raise RuntimeError(name)
        row = max(dve_ops._SUB_OPCODE_FOR_NAME.values()) + 1
        assert row < 0x20, "custom-DVE opcode rows exhausted"
        shas = {}
        for ver in ("v3", "v4"):
            s = DveOpSpec(name=name, opcode=row, uops=lower(spec, ver=ver),
                          rd1_en=_has_src1(spec))
            shas[ver] = s.sha(ver)
        op = DveOp(name, spec, subdim=False, uops_sha=shas)
        dve_ops.OPS.append(op)
        dve_ops.CUSTOM_DVE_SPECS[name] = spec
        dve_ops._SUB_OPCODE_FOR_NAME[name] = row
        return op

    # u = (z>tau0) + (z>tau1) + (z>tau2);  tau2 rides C3 (spilled to in1 [P,1])
    bucket3 = register_op(
        "BUCKET3_ANT",
        Spec(
            body=_spill_c3_to_src1(((Src0 > C0) + (Src0 > C1)) + (Src0 > C3)),
            reference=lambda in0, in1, s0, s1, imm2: (
                (in0 > s0).astype(np.float32) + (in0 > s1)
                + (in0 > np.asarray(in1, np.float32).reshape(-1, 1))
            ),
        ),
    )
    # out = (z>0) + (z>1) + (z>tau4) + (z>tau6) + u
    bucket4acc = register_op(
        "BUCKET4ACC_ANT",
        Spec(
            body=(((Src0 > Zero) + (Src0 > One))
                  + ((Src0 > C0) + (Src0 > C1))) + Src1,
            reference=lambda in0, in1, s0, s1, imm2: (
                (in0 > 0).astype(np.float32) + (in0 > 1)
                + (in0 > s0) + (in0 > s1) + in1
            ),
        ),
    )
    return bucket3, bucket4acc


def _build():
    """Trace + compile the per-core Bass kernel (once per process)."""
    if _built:
        return _built[0]

    import concourse.bacc as bacc
    import concourse.mybir as mybir
    import concourse.tile as tile

    bucket3, bucket4acc = _register_dve_ops()

    f32, f16, u8 = mybir.dt.float32, mybir.dt.float16, mybir.dt.uint8
    bf16 = mybir.dt.bfloat16
    nc = bacc.Bacc("TRN2", target_bir_lowering=False, debug=False,
                   num_devices=NCORES)

    xin_t = nc.dram_tensor("xin", [NPAIR, 128, SLAB], f16, kind="ExternalInput")
    wts_t = nc.dram_tensor("wts", [128, NTAPS, Cout], f16, kind="ExternalInput")
    nrm_t = nc.dram_tensor("nrm", [128, 7], f32, kind="ExternalInput")
    out_t = nc.dram_tensor("out", [NPAIR, 128, FOUT], u8, kind="ExternalOutput")

    with tile.TileContext(nc) as tc:
        with (
            tc.tile_pool(name="const", bufs=1) as cpool,
            tc.tile_pool(name="slab", bufs=2) as spool,
            tc.tile_pool(name="psum", bufs=4, space="PSUM") as ppool,
            tc.tile_pool(name="z", bufs=2) as zpool,
            tc.tile_pool(name="u", bufs=2) as upool,
            tc.tile_pool(name="o", bufs=2) as opool,
        ):
            # first slab piece of pair 0 goes out before anything else — it
            # gates the first real matmuls
            slab0 = spool.tile([128, SLAB], f16, tag="slab")
            nc.sync.dma_start(out=slab0[:, 0:1280], in_=xin_t.ap()[0, :, 0:1280])
            wts = cpool.tile([128, NTAPS, Cout], f16)
            nc.scalar.dma_start(out=wts[:], in_=wts_t.ap())
            nrm = cpool.tile([128, 7], f32)
            nc.scalar.dma_start(out=nrm[:], in_=nrm_t.ap())

            # PE warm-up on a zeroed tile while the first input DMA lands, so
            # the HAM clock gate opens (1.2 -> 2.4 GHz) before real matmuls.
            wu = cpool.tile([128, NCH], f16)
            nc.gpsimd.memset(wu[:], 0.0)
            wps = ppool.tile([128, 1024], f32, tag="ps")
            for _ in range(NWARM):
                nc.tensor.matmul(wps[0:64, 0:NCH], wu[0:64, 0:Cout], wu[0:64, :],
                                 tile_position=(0, 0), start=True, stop=True)

            scale, bias = nrm[:, 0:1], nrm[:, 1:2]
            tau0, tau1, tau2 = nrm[:, 2:3], nrm[:, 3:4], nrm[:, 4:5]
            tau4, tau6 = nrm[:, 5:6], nrm[:, 6:7]

            slab1 = spool.tile([128, SLAB], f16, tag="slab")
            # pieces needed after group 5 are deferred via marker deps so the
            # input-completion gate before pair-0's first ACT/DVE only covers
            # the early transfers (the gate waits for ALL in-flight DMAs)
            P0_LATE = [6656, 8704, 10752]
            P1_CUTS = [0, 4608, 9088, SLAB]
            for p in range(NPAIR):
                if p == 0:
                    slab = slab0
                    cuts = [1280, 3584]
                else:
                    slab = slab1
                    cuts = P1_CUTS
                for n, (lo, hi) in enumerate(zip(cuts[:-1], cuts[1:])):
                    eng = nc.sync if n % 2 == 0 else nc.gpsimd
                    eng.dma_start(out=slab[:, lo:hi], in_=xin_t.ap()[p, :, lo:hi])
                if p == 0:
                    # piece [3584:6656] is first needed by group 3; a marker
                    # dependent on the warm-up psum keeps it out of the
                    # startup input-completion gate without risking a stall
                    nc.scalar.copy(out=slab0[0:64, 3584:3585],
                                   in_=wps[0:64, 0:1])
                    nc.gpsimd.dma_start(out=slab0[:, 3584:6656],
                                        in_=xin_t.ap()[0, :, 3584:6656])

                z = zpool.tile([128, FOUT], f16)
                oslab = opool.tile([128, FOUT], u8)
                for j in range(NG):
                    ps = ppool.tile([128, 1024], f32, tag="ps")
                    oA = (2 * j) * NCH       # chunk 2j   flat start
                    oB = (2 * j + 1) * NCH   # chunk 2j+1 flat start
                    for t in range(NTAPS):
                        d = (t // 3) * WPAD + (t % 3)
                        st, sp = (t == 0), (t == NTAPS - 1)
                        # 4 concurrent 64x64 quadrant tiles: row = image,
                        # col = chunk parity; row groups alternate in issue
                        # order so each LDWEIGHTS overlaps the other row
                        # group's in-flight matmuls
                        nc.tensor.matmul(
                            ps[0:64, 0:NCH], wts[0:64, t, :],
                            slab[0:64, oA + d:oA + d + NCH],
                            tile_position=(0, 0), start=st, stop=sp)
                        nc.tensor.matmul(
                            ps[0:64, NCH:1024], wts[64:128, t, :],
                            slab[64:128, oA + d:oA + d + NCH],
                            tile_position=(64, 0), start=st, stop=sp)
                        nc.tensor.matmul(
                            ps[64:128, 0:NCH], wts[0:64, t, :],
                            slab[0:64, oB + d:oB + d + NCH],
                            tile_position=(0, 64), start=st, stop=sp)
                        nc.tensor.matmul(
                            ps[64:128, NCH:1024], wts[64:128, t, :],
                            slab[64:128, oB + d:oB + d + NCH],
                            tile_position=(64, 64), start=st, stop=sp)
                    # one affine pass over both banks: z fp16
                    nc.scalar.activation(
                        z[:, j * 1024:(j + 1) * 1024], ps[:],
                        mybir.ActivationFunctionType.Identity,
                        bias=bias, scale=scale)

                    # marker writes into pair-1's slab: a real dependency that
                    # keeps its input DMAs late in the schedule, so the shared
                    # DMA-completion semaphore threshold on pair-0's first
                    # ACT/DVE ops covers only pair-0's transfers
                    if p == 0 and j == 0:
                        for c in P0_LATE:
                            nc.scalar.copy(out=slab0[:, c:c + 1],
                                           in_=z[:, 0:1])
                        for c in P1_CUTS[:-1]:
                            nc.scalar.copy(out=slab1[:, c:c + 1],
                                           in_=z[:, 0:1])
                        # late pair-0 pieces, gated behind the markers
                        for n, (lo, hi) in enumerate(
                                zip(P0_LATE, P0_LATE[1:] + [SLAB])):
                            eng = nc.sync if n % 2 == 0 else nc.gpsimd
                            eng.dma_start(out=slab0[:, lo:hi],
                                          in_=xin_t.ap()[0, :, lo:hi])

                    # bucketize + store per block: small first blocks so the
                    # DVE pipeline starts early, then FD=4096 blocks to
                    # amortize the ~0.5us fixed cost per custom DVE op
                    if j + 1 in BLOCK_ENDS:
                        lo = BLOCK_ENDS[BLOCK_ENDS.index(j + 1) - 1] * 1024 \
                            if BLOCK_ENDS.index(j + 1) > 0 else 0
                        hi = (j + 1) * 1024
                        u = upool.tile([128, 4096], bf16)
                        nc.vector._custom_dve(
                            bucket3, out=u[:, 0:hi - lo], in0=z[:, lo:hi],
                            in1=tau2, s0=tau0, s1=tau1)
                        nc.vector._custom_dve(
                            bucket4acc, out=oslab[:, lo:hi],
                            in0=z[:, lo:hi], in1=u[:, 0:hi - lo],
                            s0=tau4, s1=tau6)
                        # split the store across two queues so the transfer
                        # halves run in parallel (matters for the last block,
                        # whose DMA is the kernel tail)
                        mid = (lo + hi) // 2
                        nc.sync.dma_start(out=out_t.ap()[p, :, lo:mid],
                                          in_=oslab[:, lo:mid])
                        nc.gpsimd.dma_start(out=out_t.ap()[p, :, mid:hi],
                                            in_=oslab[:, mid:hi])

    nc.compile()
    _built.append(nc)
    return nc


def _binarize_weights(w):
    """Exactly the reference's fp32 binarization. Returns (sign in {-1,0,1}, sw)."""
    w = np.asarray(w, np.float32)
    C = w.shape[0]
    wf = w.reshape(C, -1)
    bw = w - wf.mean(-1)[:, None, None, None]
    bw = bw / bw.reshape(C, -1).std(-1, ddof=1)[:, None, None, None]
    mean_abs = np.abs(bw).reshape(C, -1).mean(-1)
    sw = np.exp2(np.round(np.log2(mean_abs))).astype(np.float32)
    return np.sign(bw).astype(np.float32), sw


def kernel(x, w, lut):
    x = np.ascontiguousarray(np.asarray(x, np.float32))
    w = np.asarray(w, np.float32)
    lut = np.asarray(lut, np.float32)

    nc = _build()
    from concourse import bass_utils

    # ---- weights: binarize + fold the pow2 scale into the thresholds ----
    sgn, sw = _binarize_weights(w)                     # sgn [Cout,Cin,3,3]
    t64 = lut.astype(np.float64) / sw[:, None]         # [Cout,7] thresholds

    # lhsT per tap: wts[ci, t, co] = sgn[co, ci, dh, dw]; rows 64-127 (the
    # img-B row tiles) use the same weights
    wts = np.empty((128, NTAPS, Cout), np.float32)
    for t in range(NTAPS):
        wts[:Cin, t, :] = sgn[:, :, t // 3, t % 3].T
    wts[Cin:] = wts[:Cin]
    wts = wts.astype(np.float16)

    # ---- normalize params: z = y*s + b with tau3 -> 0, tau5 -> 1 ----
    # s>0 always; for degenerate channels (t5 == t3) use a huge power of two
    # so [z > 1] still decides [y > t3] exactly.
    t3, t5 = t64[:, 3], t64[:, 5]
    gap = t5 - t3
    s = np.where(gap > 0, 1.0 / np.where(gap > 0, gap, 1.0), 2.0 ** 100)
    bias = -t3 * s
    taus = (t64[:, [0, 1, 2, 4, 6]] - t3[:, None]) * s[:, None]
    half = np.stack([s, bias, taus[:, 0], taus[:, 1], taus[:, 2],
                     taus[:, 3], taus[:, 4]], axis=1).astype(np.float32)
    nrm = np.empty((128, 7), np.float32)
    nrm[:Cout] = half
    nrm[Cout:] = half

    # ---- fp16 zero-padded flat pair-slabs: img A on partitions 0-63, B on
    # 64-127 ----
    hi = x.astype(np.float16)                          # [32, 64, 112, 112]
    xin = np.zeros((B // 2, 128, SLAB), np.float16)    # 16 global pairs
    view = xin[:, :, :FLAT].reshape(B // 2, 128, HP, WPAD)
    view[:, 0:Cin, 1:H + 1, 1:W + 1] = hi[0::2]
    view[:, Cin:, 1:H + 1, 1:W + 1] = hi[1::2]

    # ---- run on the 8 cores (SPMD, batch-sharded) ----
    wts_np = np.ascontiguousarray(wts)
    nrm_np = np.ascontiguousarray(nrm)
    in_maps = [
        {
            "xin": np.ascontiguousarray(xin[c * NPAIR:(c + 1) * NPAIR]),
            "wts": wts_np,
            "nrm": nrm_np,
        }
        for c in range(NCORES)
    ]
    try:
        res = bass_utils.run_bass_kernel_spmd(nc, in_maps,
                                              core_ids=list(range(NCORES)))
    except Exception:
        # transient PJRT/compile hiccups happen occasionally; retry once
        res = bass_utils.run_bass_kernel_spmd(nc, in_maps,
                                              core_ids=list(range(NCORES)))
    global last_results
    last_results = res

    # ---- unshard: decode [pair, slot*64+ch, g*1024 + img*512 + w] ----
    out = np.empty((B, Cout, H, W), np.float32)
    for c in range(NCORES):
        o = res.results[c]["out"]                      # [NPAIR, 128, FOUT] u8
        # [pair, slot, ch, g, img, w] -> [pair, img, ch, g, slot, w]
        o6 = o.reshape(NPAIR, 2, Cout, NG, 2, NCH).transpose(0, 4, 2, 3, 1, 5)
        yflat = o6.reshape(NPAIR, 2, Cout, FOUT)[..., :H * WPAD]
        grid = yflat.reshape(NPAIR, 2, Cout, H, WPAD)[..., :W]
        for p in range(NPAIR):
            out[c * BPC + 2 * p] = grid[p, 0].astype(np.float32)
            out[c * BPC + 2 * p + 1] = grid[p, 1].astype(np.float32)
    return out


# revision 30
# speedup vs baseline: 1.0124x; 1.0124x over previous
"""Trainium2 Bass kernel for nn_BasicBlock_1w4a_LUT (binarized 3x3 conv + LUT bucketize).

Data-parallel over batch: 8 NeuronCores x 4 images each; no cross-core
communication. Full inputs in, full output out; shard/unshard on the host.

v2 — 4-quadrant PE tiling.

Host prep:
  - Binarize weights exactly as the reference; device weights are exactly
    +/-1 in fp16 with the pow2 scale folded into the LUT thresholds.
  - x is sent as plain fp16 (no hi/lo split: measured rel err 5e-3 << 2e-2
    budget). Each image is zero-padded into a flat 114x114 slab. A "pair
    slab" holds image A on partitions 0-63 and image B on partitions
    64-127, so the two PE row-groups work on different images and the
    input is DMA'd exactly once (halves HBM traffic vs hi/lo).
  - Per-channel affine z = y*s + b maps thresholds tau3 -> 0, tau5 -> 1
    (frees DVE scalar slots; 3 per-partition slots per custom op).

Device, per image pair:
  - The 3x3/pad-1 conv is 9 shifted dot products over the flat slab
    (junk at the 2-col row seams is computed and stripped on host).
    Output is processed in 26 chunks of 512 flat positions per image.
  - The 128x128 PE runs as FOUR 64x64 quadrant tiles (tile_position):
    row tile = image (A/B), col tile = chunk parity. Each tap round
    issues 4 concurrent K=64 matmuls, so the array is fully utilized
    with single-copy fp16 inputs — 2x the column throughput of a
    K=128 hi/lo scheme.
  - Per group (2 chunks x 2 images) the 9 tap rounds accumulate into one
    [128, 1024] PSUM tile (2 banks: img A | img B); ScalarE applies the
    per-channel affine out of PSUM in one FD=1024 pass writing z fp16.
  - Two custom DVE ops (BUCKET3 / BUCKET4ACC, registered at import into
    concourse's custom-DVE table) compute out = sum_k [z > tau_k] over
    all 7 thresholds in 2 passes per block (1-4 group blocks, small first
    for early pipeline start), writing u8. The Vector engine stream is the
    overall bottleneck (~64us of the ~86us span).
  - Input DMA pieces not needed until later groups are deferred behind
    marker-copy dependencies: every in-flight DMA counts toward the shared
    DMA-completion semaphore, so early consumers would otherwise gate on
    pair-1's transfers (~10us of extra startup latency).
"""

import numpy as np

# ---- problem constants (hardcoded per contract) ----
B, Cin, Cout, H, W = 32, 64, 64, 112, 112
NCORES = 8
BPC = B // NCORES          # images per core
NPAIR = BPC // 2           # image pairs per core (2)
HP = H + 2                 # 114
WPAD = W + 2               # 114
FLAT = HP * WPAD           # 12996 real slab elements
SLAB = 13056               # padded slab alloc (max rhs read 12799+230)
NCH = 512                  # flat positions per chunk (one PSUM bank)
FOUT = 12800               # flat out positions per image (group0 512 + 12*1024)
NG = 13                    # groups per pair; group 0 is half-width (N=256)
NTAPS = 9
NWARM = 9                  # PE warm-up matmuls
BLOCK_ENDS = [1, 2, 3, 5, 7, 9, 13]  # DVE block group-boundaries per pair
VALID = (H - 1) * WPAD + W # 12766 flat positions that contain real pixels

_built = []
last_results = None


def _register_dve_ops():
    from concourse.dve_spec import (
        Spec, Src0, Src1, C0, C1, C3, One, Zero, lower,
        _spill_c3_to_src1, _has_src1,
    )
    import concourse.dve_ops as dve_ops
    from concourse.dve_ops import DveOp
    from concourse.dve_uop import DveOpSpec

    def register_op(name, spec):
        if name in dve_ops._SUB_OPCODE_FOR_NAME:
            for op in dve_ops.OPS:
                if op.name == name:
                    return op
            raise RuntimeError(name)
        row = max(dve_ops._SUB_OPCODE_FOR_NAME.values()) + 1
        assert row < 0x20, "custom-DVE opcode rows exhausted"
        shas = {}
        for ver in ("v3", "v4"):
            s = DveOpSpec(name=name, opcode=row, uops=lower(spec, ver=ver),
                          rd1_en=_has_src1(spec))
            shas[ver] = s.sha(ver)
        op = DveOp(name, spec, subdim=False, uops_sha=shas)
        dve_ops.OPS.append(op)
        dve_ops.CUSTOM_DVE_SPECS[name] = spec
        dve_ops._SUB_OPCODE_FOR_NAME[name] = row
        return op

    # u = (z>tau0) + (z>tau1) + (z>tau2);  tau2 rides C3 (spilled to in1 [P,1])
    bucket3 = register_op(
        "BUCKET3_ANT",
        Spec(
            body=_spill_c3_to_src1(((Src0 > C0) + (Src0 > C1)) + (Src0 > C3)),
            reference=lambda in0, in1, s0, s1, imm2: (
                (in0 > s0).astype(np.float32) + (in0 > s1)
                + (in0 > np.asarray(in1, np.float32).reshape(-1, 1))
            ),
        ),
    )
    # out = (z>0) + (z>1) + (z>tau4) + (z>tau6) + u
    bucket4acc = register_op(
        "BUCKET4ACC_ANT",
        Spec(
            body=(((Src0 > Zero) + (Src0 > One))
                  + ((Src0 > C0) + (Src0 > C1))) + Src1,
            reference=lambda in0, in1, s0, s1, imm2: (
                (in0 > 0).astype(np.float32) + (in0 > 1)
                + (in0 > s0) + (in0 > s1) + in1
            ),
        ),
    )
    return bucket3, bucket4acc


def _build():
    """Trace + compile the per-core Bass kernel (once per process)."""
    if _built:
        return _built[0]

    import concourse.bacc as bacc
    import concourse.mybir as mybir
    import concourse.tile as tile

    bucket3, bucket4acc = _register_dve_ops()

    f32, f16, u8 = mybir.dt.float32, mybir.dt.float16, mybir.dt.uint8
    bf16 = mybir.dt.bfloat16
    nc = bacc.Bacc("TRN2", target_bir_lowering=False, debug=False,
                   num_devices=NCORES)

    xin_t = nc.dram_tensor("xin", [NPAIR, 128, SLAB], f16, kind="ExternalInput")
    wts_t = nc.dram_tensor("wts", [128, NTAPS, Cout], f16, kind="ExternalInput")
    nrm_t = nc.dram_tensor("nrm", [128, 7], f32, kind="ExternalInput")
    out_t = nc.dram_tensor("out", [NPAIR, 128, FOUT], u8, kind="ExternalOutput")

    with tile.TileContext(nc) as tc:
        with (
            tc.tile_pool(name="const", bufs=1) as cpool,
            tc.tile_pool(name="slab", bufs=2) as spool,
            tc.tile_pool(name="psum", bufs=4, space="PSUM") as ppool,
            tc.tile_pool(name="z", bufs=2) as zpool,
            tc.tile_pool(name="u", bufs=2) as upool,
            tc.tile_pool(name="o", bufs=2) as opool,
        ):
            # first slab piece of pair 0 goes out before anything else — it
            # gates the first real matmuls
            slab0 = spool.tile([128, SLAB], f16, tag="slab")
            nc.sync.dma_start(out=slab0[:, 0:1280], in_=xin_t.ap()[0, :, 0:1280])
            wts = cpool.tile([128, NTAPS, Cout], f16)
            nc.scalar.dma_start(out=wts[:], in_=wts_t.ap())
            nrm = cpool.tile([128, 7], f32)
            nc.scalar.dma_start(out=nrm[:], in_=nrm_t.ap())

            # PE warm-up on a zeroed tile while the first input DMA lands, so
            # the HAM clock gate opens (1.2 -> 2.4 GHz) before real matmuls.
            wu = cpool.tile([128, NCH], f16)
            nc.gpsimd.memset(wu[:], 0.0)
            wps = ppool.tile([128, 1024], f32, tag="ps")
            for _ in range(NWARM):
                nc.tensor.matmul(wps[0:64, 0:NCH], wu[0:64, 0:Cout], wu[0:64, :],
                                 tile_position=(0, 0), start=True, stop=True)

            scale, bias = nrm[:, 0:1], nrm[:, 1:2]
            tau0, tau1, tau2 = nrm[:, 2:3], nrm[:, 3:4], nrm[:, 4:5]
            tau4, tau6 = nrm[:, 5:6], nrm[:, 6:7]

            slab1 = spool.tile([128, SLAB], f16, tag="slab")
            # pieces needed after group 5 are deferred via marker deps so the
            # input-completion gate before pair-0's first ACT/DVE only covers
            # the early transfers (the gate waits for ALL in-flight DMAs)
            P0_LATE = [6656, 8704, 10752]
            P1_CUTS = [0, 4608, 9088, SLAB]
            for p in range(NPAIR):
                if p == 0:
                    slab = slab0
                    cuts = [1280, 3584]
                else:
                    slab = slab1
                    cuts = P1_CUTS
                for n, (lo, hi) in enumerate(zip(cuts[:-1], cuts[1:])):
                    eng = nc.sync if n % 2 == 0 else nc.gpsimd
                    eng.dma_start(out=slab[:, lo:hi], in_=xin_t.ap()[p, :, lo:hi])
                if p == 0:
                    # piece [3584:6656] is first needed by group 3; a marker
                    # dependent on the warm-up psum keeps it out of the
                    # startup input-completion gate without risking a stall
                    nc.scalar.copy(out=slab0[0:64, 3584:3585],
                                   in_=wps[0:64, 0:1])
                    nc.gpsimd.dma_start(out=slab0[:, 3584:6656],
                                        in_=xin_t.ap()[0, :, 3584:6656])

                z = zpool.tile([128, FOUT], f16)
                oslab = opool.tile([128, FOUT], u8)
                for j in range(NG):
                    ps = ppool.tile([128, 1024], f32, tag="ps")
                    # group 0 is half-width so the first ACT/DVE block is
                    # ready ~1.5us earlier; it fits one PSUM bank
                    n = 256 if j == 0 else NCH
                    boff = NCH  # img-B tiles always write bank 1
                    zlo = 0 if j == 0 else 512 + (j - 1) * 1024
                    oA = zlo                 # first chunk flat start
                    oB = zlo + n             # second chunk flat start
                    for t in range(NTAPS):
                        d = (t // 3) * WPAD + (t % 3)
                        st, sp = (t == 0), (t == NTAPS - 1)
                        # 4 concurrent 64x64 quadrant tiles: row = image,
                        # col = chunk parity; row groups alternate in issue
                        # order so each LDWEIGHTS overlaps the other row
                        # group's in-flight matmuls
                        nc.tensor.matmul(
                            ps[0:64, 0:n], wts[0:64, t, :],
                            slab[0:64, oA + d:oA + d + n],
                            tile_position=(0, 0), start=st, stop=sp)
                        nc.tensor.matmul(
                            ps[0:64, boff:boff + n], wts[64:128, t, :],
                            slab[64:128, oA + d:oA + d + n],
                            tile_position=(64, 0), start=st, stop=sp)
                        nc.tensor.matmul(
                            ps[64:128, 0:n], wts[0:64, t, :],
                            slab[0:64, oB + d:oB + d + n],
                            tile_position=(0, 64), start=st, stop=sp)
                        nc.tensor.matmul(
                            ps[64:128, boff:boff + n], wts[64:128, t, :],
                            slab[64:128, oB + d:oB + d + n],
                            tile_position=(64, 64), start=st, stop=sp)
                    # one affine pass over both banks: z fp16
                    act_src = (ps[:] if j != 0 else
                               ps[:].rearrange("p (b w) -> p b w", w=NCH)
                               [:, :, 0:256])
                    nc.scalar.activation(
                        z[:, zlo:zlo + 2 * n], act_src,
                        mybir.ActivationFunctionType.Identity,
                        bias=bias, scale=scale)

                    # marker writes into pair-1's slab: a real dependency that
                    # keeps its input DMAs late in the schedule, so the shared
                    # DMA-completion semaphore threshold on pair-0's first
                    # ACT/DVE ops covers only pair-0's transfers
                    if p == 0 and j == 0:
                        for c in P0_LATE:
                            nc.scalar.copy(out=slab0[:, c:c + 1],
                                           in_=z[:, 0:1])
                        for c in P1_CUTS[:-1]:
                            nc.scalar.copy(out=slab1[:, c:c + 1],
                                           in_=z[:, 0:1])
                        # late pair-0 pieces, gated behind the markers
                        for n, (lo, hi) in enumerate(
                                zip(P0_LATE, P0_LATE[1:] + [SLAB])):
                            eng = nc.sync if n % 2 == 0 else nc.gpsimd
                            eng.dma_start(out=slab0[:, lo:hi],
                                          in_=xin_t.ap()[0, :, lo:hi])

                    # bucketize + store per block: small first blocks so the
                    # DVE pipeline starts early, then FD=4096 blocks to
                    # amortize the ~0.5us fixed cost per custom DVE op
                    if j + 1 in BLOCK_ENDS:
                        def _zoff(g):
                            return 0 if g == 0 else 512 + (g - 1) * 1024
                        bi = BLOCK_ENDS.index(j + 1)
                        lo = _zoff(BLOCK_ENDS[bi - 1]) if bi > 0 else 0
                        hi = _zoff(j + 1)
                        u = upool.tile([128, 4096], bf16)
                        nc.vector._custom_dve(
                            bucket3, out=u[:, 0:hi - lo], in0=z[:, lo:hi],
                            in1=tau2, s0=tau0, s1=tau1)
                        nc.vector._custom_dve(
                            bucket4acc, out=oslab[:, lo:hi],
                            in0=z[:, lo:hi], in1=u[:, 0:hi - lo],
                            s0=tau4, s1=tau6)
                        # split the store across two queues so the transfer
                        # halves run in parallel (matters for the last block,
                        # whose DMA is the kernel tail)
                        mid = (lo + hi) // 2
                        nc.sync.dma_start(out=out_t.ap()[p, :, lo:mid],
                                          in_=oslab[:, lo:mid])
                        nc.gpsimd.dma_start(out=out_t.ap()[p, :, mid:hi],
                                            in_=oslab[:, mid:hi])

    nc.compile()
    _built.append(nc)
    return nc


def _binarize_weights(w):
    """Exactly the reference's fp32 binarization. Returns (sign in {-1,0,1}, sw)."""
    w = np.asarray(w, np.float32)
    C = w.shape[0]
    wf = w.reshape(C, -1)
    bw = w - wf.mean(-1)[:, None, None, None]
    bw = bw / bw.reshape(C, -1).std(-1, ddof=1)[:, None, None, None]
    mean_abs = np.abs(bw).reshape(C, -1).mean(-1)
    sw = np.exp2(np.round(np.log2(mean_abs))).astype(np.float32)
    return np.sign(bw).astype(np.float32), sw


def kernel(x, w, lut):
    x = np.ascontiguousarray(np.asarray(x, np.float32))
    w = np.asarray(w, np.float32)
    lut = np.asarray(lut, np.float32)

    nc = _build()
    from concourse import bass_utils

    # ---- weights: binarize + fold the pow2 scale into the thresholds ----
    sgn, sw = _binarize_weights(w)                     # sgn [Cout,Cin,3,3]
    t64 = lut.astype(np.float64) / sw[:, None]         # [Cout,7] thresholds

    # lhsT per tap: wts[ci, t, co] = sgn[co, ci, dh, dw]; rows 64-127 (the
    # img-B row tiles) use the same weights
    wts = np.empty((128, NTAPS, Cout), np.float32)
    for t in range(NTAPS):
        wts[:Cin, t, :] = sgn[:, :, t // 3, t % 3].T
    wts[Cin:] = wts[:Cin]
    wts = wts.astype(np.float16)

    # ---- normalize params: z = y*s + b with tau3 -> 0, tau5 -> 1 ----
    # s>0 always; for degenerate channels (t5 == t3) use a huge power of two
    # so [z > 1] still decides [y > t3] exactly.
    t3, t5 = t64[:, 3], t64[:, 5]
    gap = t5 - t3
    s = np.where(gap > 0, 1.0 / np.where(gap > 0, gap, 1.0), 2.0 ** 100)
    bias = -t3 * s
    taus = (t64[:, [0, 1, 2, 4, 6]] - t3[:, None]) * s[:, None]
    half = np.stack([s, bias, taus[:, 0], taus[:, 1], taus[:, 2],
                     taus[:, 3], taus[:, 4]], axis=1).astype(np.float32)
    nrm = np.empty((128, 7), np.float32)
    nrm[:Cout] = half
    nrm[Cout:] = half

    # ---- fp16 zero-padded flat pair-slabs: img A on partitions 0-63, B on
    # 64-127 ----
    hi = x.astype(np.float16)                          # [32, 64, 112, 112]
    xin = np.zeros((B // 2, 128, SLAB), np.float16)    # 16 global pairs
    view = xin[:, :, :FLAT].reshape(B // 2, 128, HP, WPAD)
    view[:, 0:Cin, 1:H + 1, 1:W + 1] = hi[0::2]
    view[:, Cin:, 1:H + 1, 1:W + 1] = hi[1::2]

    # ---- run on the 8 cores (SPMD, batch-sharded) ----
    wts_np = np.ascontiguousarray(wts)
    nrm_np = np.ascontiguousarray(nrm)
    in_maps = [
        {
            "xin": np.ascontiguousarray(xin[c * NPAIR:(c + 1) * NPAIR]),
            "wts": wts_np,
            "nrm": nrm_np,
        }
        for c in range(NCORES)
    ]
    try:
        res = bass_utils.run_bass_kernel_spmd(nc, in_maps,
                                              core_ids=list(range(NCORES)))
    except Exception:
        # transient PJRT/compile hiccups happen occasionally; retry once
        res = bass_utils.run_bass_kernel_spmd(nc, in_maps,
                                              core_ids=list(range(NCORES)))
    global last_results
    last_results = res

    # ---- unshard: group 0 is half-width (slots of 256), groups 1-12 are
    # [g, img, slot, w] with 512-wide slots ----
    out = np.empty((B, Cout, H, W), np.float32)
    for c in range(NCORES):
        o = res.results[c]["out"]                      # [NPAIR, 128, FOUT] u8
        # group 0: cols [0:512] = [slot, ch] x [img, w(256)]
        o0 = o[:, :, 0:512].reshape(NPAIR, 2, Cout, 2, 256)
        p0 = o0.transpose(0, 3, 2, 1, 4).reshape(NPAIR, 2, Cout, 512)
        # groups 1-12: cols [512:] = [slot, ch] x [g, img, w(512)]
        orest = o[:, :, 512:].reshape(NPAIR, 2, Cout, NG - 1, 2, NCH)
        prest = orest.transpose(0, 4, 2, 3, 1, 5).reshape(
            NPAIR, 2, Cout, (NG - 1) * 1024)
        yflat = np.concatenate([p0, prest], axis=-1)[..., :H * WPAD]
        grid = yflat.reshape(NPAIR, 2, Cout, H, WPAD)[..., :W]
        for p in range(NPAIR):
            out[c * BPC + 2 * p] = grid[p, 0].astype(np.float32)
            out[c * BPC + 2 * p + 1] = grid[p, 1].astype(np.float32)
    return out


# revision 34
# speedup vs baseline: 1.0126x; 1.0001x over previous
"""Trainium2 Bass kernel for nn_BasicBlock_1w4a_LUT (binarized 3x3 conv + LUT bucketize).

Data-parallel over batch: 8 NeuronCores x 4 images each; no cross-core
communication. Full inputs in, full output out; shard/unshard on the host.

v2 — 4-quadrant PE tiling.

Host prep:
  - Binarize weights exactly as the reference; device weights are exactly
    +/-1 in fp16 with the pow2 scale folded into the LUT thresholds.
  - x is sent as plain fp16 (no hi/lo split: measured rel err 5e-3 << 2e-2
    budget). Each image is zero-padded into a flat 114x114 slab. A "pair
    slab" holds image A on partitions 0-63 and image B on partitions
    64-127, so the two PE row-groups work on different images and the
    input is DMA'd exactly once (halves HBM traffic vs hi/lo).
  - Per-channel affine z = y*s + b maps thresholds tau3 -> 0, tau5 -> 1
    (frees DVE scalar slots; 3 per-partition slots per custom op).

Device, per image pair:
  - The 3x3/pad-1 conv is 9 shifted dot products over the flat slab
    (junk at the 2-col row seams is computed and stripped on host).
    Output is processed in 26 chunks of 512 flat positions per image.
  - The 128x128 PE runs as FOUR 64x64 quadrant tiles (tile_position):
    row tile = image (A/B), col tile = chunk parity. Each tap round
    issues 4 concurrent K=64 matmuls, so the array is fully utilized
    with single-copy fp16 inputs — 2x the column throughput of a
    K=128 hi/lo scheme.
  - Per group (2 chunks x 2 images) the 9 tap rounds accumulate into one
    [128, 1024] PSUM tile (2 banks: img A | img B); ScalarE applies the
    per-channel affine out of PSUM in one FD=1024 pass writing z fp16.
  - Two custom DVE ops (BUCKET3 / BUCKET4ACC, registered at import into
    concourse's custom-DVE table) compute out = sum_k [z > tau_k] over
    all 7 thresholds in 2 passes per block (1-4 group blocks, small first
    for early pipeline start), writing u8. The Vector engine stream is the
    overall bottleneck (~64us of the ~86us span).
  - Input DMA pieces not needed until later groups are deferred behind
    marker-copy dependencies: every in-flight DMA counts toward the shared
    DMA-completion semaphore, so early consumers would otherwise gate on
    pair-1's transfers (~10us of extra startup latency).
"""

import numpy as np

# ---- problem constants (hardcoded per contract) ----
B, Cin, Cout, H, W = 32, 64, 64, 112, 112
NCORES = 8
BPC = B // NCORES          # images per core
NPAIR = BPC // 2           # image pairs per core (2)
HP = H + 2                 # 114
WPAD = W + 2               # 114
FLAT = HP * WPAD           # 12996 real slab elements
SLAB = 13056               # padded slab alloc (max rhs read 12799+230)
NCH = 512                  # flat positions per chunk (one PSUM bank)
FOUT = 12800               # flat out positions per image (group0 512 + 12*1024)
NG = 13                    # groups per pair; group 0 is half-width (N=256)
NTAPS = 9
NWARM = 9                  # PE warm-up matmuls
BLOCK_ENDS = [1, 2, 3, 5, 7, 9, 13]  # DVE block group-boundaries per pair
VALID = (H - 1) * WPAD + W # 12766 flat positions that contain real pixels

_built = []
last_results = None


def _register_dve_ops():
    from concourse.dve_spec import (
        Spec, Src0, Src1, C0, C1, C3, One, Zero, lower,
        _spill_c3_to_src1, _has_src1,
    )
    import concourse.dve_ops as dve_ops
    from concourse.dve_ops import DveOp
    from concourse.dve_uop import DveOpSpec

    def register_op(name, spec):
        if name in dve_ops._SUB_OPCODE_FOR_NAME:
            for op in dve_ops.OPS:
                if op.name == name:
                    return op
            raise RuntimeError(name)
        row = max(dve_ops._SUB_OPCODE_FOR_NAME.values()) + 1
        assert row < 0x20, "custom-DVE opcode rows exhausted"
        shas = {}
        for ver in ("v3", "v4"):
            s = DveOpSpec(name=name, opcode=row, uops=lower(spec, ver=ver),
                          rd1_en=_has_src1(spec))
            shas[ver] = s.sha(ver)
        op = DveOp(name, spec, subdim=False, uops_sha=shas)
        dve_ops.OPS.append(op)
        dve_ops.CUSTOM_DVE_SPECS[name] = spec
        dve_ops._SUB_OPCODE_FOR_NAME[name] = row
        return op

    # u = (z>tau0) + (z>tau1) + (z>tau2);  tau2 rides C3 (spilled to in1 [P,1])
    bucket3 = register_op(
        "BUCKET3_ANT",
        Spec(
            body=_spill_c3_to_src1(((Src0 > C0) + (Src0 > C1)) + (Src0 > C3)),
            reference=lambda in0, in1, s0, s1, imm2: (
                (in0 > s0).astype(np.float32) + (in0 > s1)
                + (in0 > np.asarray(in1, np.float32).reshape(-1, 1))
            ),
        ),
    )
    # out = (z>0) + (z>1) + (z>tau4) + (z>tau6) + u
    bucket4acc = register_op(
        "BUCKET4ACC_ANT",
        Spec(
            body=(((Src0 > Zero) + (Src0 > One))
                  + ((Src0 > C0) + (Src0 > C1))) + Src1,
            reference=lambda in0, in1, s0, s1, imm2: (
                (in0 > 0).astype(np.float32) + (in0 > 1)
                + (in0 > s0) + (in0 > s1) + in1
            ),
        ),
    )
    return bucket3, bucket4acc


def _build():
    """Trace + compile the per-core Bass kernel (once per process)."""
    if _built:
        return _built[0]

    import concourse.bacc as bacc
    import concourse.mybir as mybir
    import concourse.tile as tile

    bucket3, bucket4acc = _register_dve_ops()

    f32, f16, u8 = mybir.dt.float32, mybir.dt.float16, mybir.dt.uint8
    bf16 = mybir.dt.bfloat16
    nc = bacc.Bacc("TRN2", target_bir_lowering=False, debug=False,
                   num_devices=NCORES)

    xin_t = nc.dram_tensor("xin", [NPAIR, 128, SLAB], f16, kind="ExternalInput")
    wts_t = nc.dram_tensor("wts", [128, NTAPS, Cout], f16, kind="ExternalInput")
    nrm_t = nc.dram_tensor("nrm", [128, 7], f32, kind="ExternalInput")
    out_t = nc.dram_tensor("out", [NPAIR, 128, FOUT], u8, kind="ExternalOutput")

    with tile.TileContext(nc) as tc:
        with (
            tc.tile_pool(name="const", bufs=1) as cpool,
            tc.tile_pool(name="slab", bufs=2) as spool,
            tc.tile_pool(name="psum", bufs=4, space="PSUM") as ppool,
            tc.tile_pool(name="z", bufs=2) as zpool,
            tc.tile_pool(name="u", bufs=2) as upool,
            tc.tile_pool(name="o", bufs=2) as opool,
        ):
            # first slab piece of pair 0 goes out before anything else — it
            # gates the first real matmuls
            slab0 = spool.tile([128, SLAB], f16, tag="slab")
            nc.sync.dma_start(out=slab0[:, 0:1280], in_=xin_t.ap()[0, :, 0:1280])
            wts = cpool.tile([128, NTAPS, Cout], f16)
            nc.scalar.dma_start(out=wts[:], in_=wts_t.ap())
            nrm = cpool.tile([128, 7], f32)
            nc.scalar.dma_start(out=nrm[:], in_=nrm_t.ap())

            # PE warm-up on a zeroed tile while the first input DMA lands, so
            # the HAM clock gate opens (1.2 -> 2.4 GHz) before real matmuls.
            wu = cpool.tile([128, NCH], f16)
            nc.vector.memset(wu[:], 0.0)
            wps = ppool.tile([128, 1024], f32, tag="ps")
            for _ in range(NWARM):
                nc.tensor.matmul(wps[0:64, 0:NCH], wu[0:64, 0:Cout], wu[0:64, :],
                                 tile_position=(0, 0), start=True, stop=True)

            scale, bias = nrm[:, 0:1], nrm[:, 1:2]
            tau0, tau1, tau2 = nrm[:, 2:3], nrm[:, 3:4], nrm[:, 4:5]
            tau4, tau6 = nrm[:, 5:6], nrm[:, 6:7]

            slab1 = spool.tile([128, SLAB], f16, tag="slab")
            # pieces needed after group 5 are deferred via marker deps so the
            # input-completion gate before pair-0's first ACT/DVE only covers
            # the early transfers (the gate waits for ALL in-flight DMAs)
            P0_LATE = [6656, 8704, 10752]
            P1_CUTS = [0, 4608, 9088, SLAB]
            for p in range(NPAIR):
                if p == 0:
                    slab = slab0
                    cuts = [1280, 3584]
                else:
                    slab = slab1
                    cuts = P1_CUTS
                for n, (lo, hi) in enumerate(zip(cuts[:-1], cuts[1:])):
                    eng = nc.sync if n % 2 == 0 else nc.gpsimd
                    eng.dma_start(out=slab[:, lo:hi], in_=xin_t.ap()[p, :, lo:hi])
                if p == 0:
                    # piece [3584:6656] is first needed by group 3; a marker
                    # dependent on the warm-up psum keeps it out of the
                    # startup input-completion gate without risking a stall
                    nc.scalar.copy(out=slab0[0:64, 3584:3585],
                                   in_=wps[0:64, 0:1])
                    nc.gpsimd.dma_start(out=slab0[:, 3584:6656],
                                        in_=xin_t.ap()[0, :, 3584:6656])

                z = zpool.tile([128, FOUT], f16)
                oslab = opool.tile([128, FOUT], u8)
                for j in range(NG):
                    ps = ppool.tile([128, 1024], f32, tag="ps")
                    # group 0 is half-width so the first ACT/DVE block is
                    # ready ~1.5us earlier; it fits one PSUM bank
                    n = 256 if j == 0 else NCH
                    boff = NCH  # img-B tiles always write bank 1
                    zlo = 0 if j == 0 else 512 + (j - 1) * 1024
                    oA = zlo                 # first chunk flat start
                    oB = zlo + n             # second chunk flat start
                    for t in range(NTAPS):
                        d = (t // 3) * WPAD + (t % 3)
                        st, sp = (t == 0), (t == NTAPS - 1)
                        # 4 concurrent 64x64 quadrant tiles: row = image,
                        # col = chunk parity; row groups alternate in issue
                        # order so each LDWEIGHTS overlaps the other row
                        # group's in-flight matmuls
                        nc.tensor.matmul(
                            ps[0:64, 0:n], wts[0:64, t, :],
                            slab[0:64, oA + d:oA + d + n],
                            tile_position=(0, 0), start=st, stop=sp)
                        nc.tensor.matmul(
                            ps[0:64, boff:boff + n], wts[64:128, t, :],
                            slab[64:128, oA + d:oA + d + n],
                            tile_position=(64, 0), start=st, stop=sp)
                        nc.tensor.matmul(
                            ps[64:128, 0:n], wts[0:64, t, :],
                            slab[0:64, oB + d:oB + d + n],
                            tile_position=(0, 64), start=st, stop=sp)
                        nc.tensor.matmul(
                            ps[64:128, boff:boff + n], wts[64:128, t, :],
                            slab[64:128, oB + d:oB + d + n],
                            tile_position=(64, 64), start=st, stop=sp)
                    # one affine pass over both banks: z fp16
                    act_src = (ps[:] if j != 0 else
                               ps[:].rearrange("p (b w) -> p b w", w=NCH)
                               [:, :, 0:256])
                    nc.scalar.activation(
                        z[:, zlo:zlo + 2 * n], act_src,
                        mybir.ActivationFunctionType.Identity,
                        bias=bias, scale=scale)

                    # marker writes into pair-1's slab: a real dependency that
                    # keeps its input DMAs late in the schedule, so the shared
                    # DMA-completion semaphore threshold on pair-0's first
                    # ACT/DVE ops covers only pair-0's transfers
                    if p == 0 and j == 0:
                        # markers ride GpSimd (idle here) so they don't delay
                        # the Scalar queue's ACT chain
                        for c in P0_LATE:
                            nc.gpsimd.tensor_copy(slab0[:, c:c + 1], z[:, 0:1])
                        for c in P1_CUTS[:-1]:
                            nc.gpsimd.tensor_copy(slab1[:, c:c + 1], z[:, 0:1])
                        # late pair-0 pieces, gated behind the markers
                        for n, (lo, hi) in enumerate(
                                zip(P0_LATE, P0_LATE[1:] + [SLAB])):
                            eng = nc.sync if n % 2 == 0 else nc.gpsimd
                            eng.dma_start(out=slab0[:, lo:hi],
                                          in_=xin_t.ap()[0, :, lo:hi])

                    # bucketize + store per block: small first blocks so the
                    # DVE pipeline starts early, then FD=4096 blocks to
                    # amortize the ~0.5us fixed cost per custom DVE op
                    if j + 1 in BLOCK_ENDS:
                        def _zoff(g):
                            return 0 if g == 0 else 512 + (g - 1) * 1024
                        bi = BLOCK_ENDS.index(j + 1)
                        lo = _zoff(BLOCK_ENDS[bi - 1]) if bi > 0 else 0
                        hi = _zoff(j + 1)
                        u = upool.tile([128, 4096], bf16)
                        nc.vector._custom_dve(
                            bucket3, out=u[:, 0:hi - lo], in0=z[:, lo:hi],
                            in1=tau2, s0=tau0, s1=tau1)
                        nc.vector._custom_dve(
                            bucket4acc, out=oslab[:, lo:hi],
                            in0=z[:, lo:hi], in1=u[:, 0:hi - lo],
                            s0=tau4, s1=tau6)
                        # split the store across two queues so the transfer
                        # halves run in parallel (matters for the last block,
                        # whose DMA is the kernel tail)
                        mid = (lo + hi) // 2
                        nc.sync.dma_start(out=out_t.ap()[p, :, lo:mid],
                                          in_=oslab[:, lo:mid])
                        nc.gpsimd.dma_start(out=out_t.ap()[p, :, mid:hi],
                                            in_=oslab[:, mid:hi])

    nc.compile()
    _built.append(nc)
    return nc


def _binarize_weights(w):
    """Exactly the reference's fp32 binarization. Returns (sign in {-1,0,1}, sw)."""
    w = np.asarray(w, np.float32)
    C = w.shape[0]
    wf = w.reshape(C, -1)
    bw = w - wf.mean(-1)[:, None, None, None]
    bw = bw / bw.reshape(C, -1).std(-1, ddof=1)[:, None, None, None]
    mean_abs = np.abs(bw).reshape(C, -1).mean(-1)
    sw = np.exp2(np.round(np.log2(mean_abs))).astype(np.float32)
    return np.sign(bw).astype(np.float32), sw


def kernel(x, w, lut):
    x = np.ascontiguousarray(np.asarray(x, np.float32))
    w = np.asarray(w, np.float32)
    lut = np.asarray(lut, np.float32)

    nc = _build()
    from concourse import bass_utils

    # ---- weights: binarize + fold the pow2 scale into the thresholds ----
    sgn, sw = _binarize_weights(w)                     # sgn [Cout,Cin,3,3]
    t64 = lut.astype(np.float64) / sw[:, None]         # [Cout,7] thresholds

    # lhsT per tap: wts[ci, t, co] = sgn[co, ci, dh, dw]; rows 64-127 (the
    # img-B row tiles) use the same weights
    wts = np.empty((128, NTAPS, Cout), np.float32)
    for t in range(NTAPS):
        wts[:Cin, t, :] = sgn[:, :, t // 3, t % 3].T
    wts[Cin:] = wts[:Cin]
    wts = wts.astype(np.float16)

    # ---- normalize params: z = y*s + b with tau3 -> 0, tau5 -> 1 ----
    # s>0 always; for degenerate channels (t5 == t3) use a huge power of two
    # so [z > 1] still decides [y > t3] exactly.
    t3, t5 = t64[:, 3], t64[:, 5]
    gap = t5 - t3
    s = np.where(gap > 0, 1.0 / np.where(gap > 0, gap, 1.0), 2.0 ** 100)
    bias = -t3 * s
    taus = (t64[:, [0, 1, 2, 4, 6]] - t3[:, None]) * s[:, None]
    half = np.stack([s, bias, taus[:, 0], taus[:, 1], taus[:, 2],
                     taus[:, 3], taus[:, 4]], axis=1).astype(np.float32)
    nrm = np.empty((128, 7), np.float32)
    nrm[:Cout] = half
    nrm[Cout:] = half

    # ---- fp16 zero-padded flat pair-slabs: img A on partitions 0-63, B on
    # 64-127 ----
    hi = x.astype(np.float16)                          # [32, 64, 112, 112]
    xin = np.zeros((B // 2, 128, SLAB), np.float16)    # 16 global pairs
    view = xin[:, :, :FLAT].reshape(B // 2, 128, HP, WPAD)
    view[:, 0:Cin, 1:H + 1, 1:W + 1] = hi[0::2]
    view[:, Cin:, 1:H + 1, 1:W + 1] = hi[1::2]

    # ---- run on the 8 cores (SPMD, batch-sharded) ----
    wts_np = np.ascontiguousarray(wts)
    nrm_np = np.ascontiguousarray(nrm)
    in_maps = [
        {
            "xin": np.ascontiguousarray(xin[c * NPAIR:(c + 1) * NPAIR]),
            "wts": wts_np,
            "nrm": nrm_np,
        }
        for c in range(NCORES)
    ]
    try:
        res = bass_utils.run_bass_kernel_spmd(nc, in_maps,
                                              core_ids=list(range(NCORES)))
    except Exception:
        # transient PJRT/compile hiccups happen occasionally; retry once
        res = bass_utils.run_bass_kernel_spmd(nc, in_maps,
                                              core_ids=list(range(NCORES)))
    global last_results
    last_results = res

    # ---- unshard: group 0 is half-width (slots of 256), groups 1-12 are
    # [g, img, slot, w] with 512-wide slots ----
    out = np.empty((B, Cout, H, W), np.float32)
    for c in range(NCORES):
        o = res.results[c]["out"]                      # [NPAIR, 128, FOUT] u8
        # group 0: cols [0:512] = [slot, ch] x [img, w(256)]
        o0 = o[:, :, 0:512].reshape(NPAIR, 2, Cout, 2, 256)
        p0 = o0.transpose(0, 3, 2, 1, 4).reshape(NPAIR, 2, Cout, 512)
        # groups 1-12: cols [512:] = [slot, ch] x [g, img, w(512)]
        orest = o[:, :, 512:].reshape(NPAIR, 2, Cout, NG - 1, 2, NCH)
        prest = orest.transpose(0, 4, 2, 3, 1, 5).reshape(
            NPAIR, 2, Cout, (NG - 1) * 1024)
        yflat = np.concatenate([p0, prest], axis=-1)[..., :H * WPAD]
        grid = yflat.reshape(NPAIR, 2, Cout, H, WPAD)[..., :W]
        for p in range(NPAIR):
            out[c * BPC + 2 * p] = grid[p, 0].astype(np.float32)
            out[c * BPC + 2 * p + 1] = grid[p, 1].astype(np.float32)
    return out
